# revision 42
# baseline (speedup 1.0000x reference)
"""NT-Xent loss Trainium2 kernel (8-core SPMD, Bass/Tile).

Math: loss = mean_a [ log(den_a) - pos_a/tau ],
  den_a = sum_{b != a} exp(sim_ab/tau),  sim = Z Z^T,  Z = row-normalized
  concat(e_i, e_j).

Sharding: row-parallel over the 8192 rows of the similarity matrix. Every
core receives the full embedding matrix rotated by -c*1024 rows so that its
1024 rows are always local rows 0..1023 (identical SPMD program on all
cores). Each core computes row sums of exp(sim/tau) for its rows against
all 8192 columns (fused exp+accumulate on the scalar engine), plus the
per-row self-similarity (z2) and positive-pair dot (pos). The host performs
the final gather: den = rowsum - exp(z2/tau), loss = mean(log den - pos/tau).

Engine budget per core: ACT does 8.4M exp (the bottleneck, ~66us); PE does
the 1024x8192x128 fp32r matmul plus 64 transposes; DVE does normalization
(squares, row reductions, a magic-constant+Newton rsqrt -- keeps ACT free
of Sqrt/Ln table loads), scaling, and the PSUM->SBUF float32r copies.

Note: tensor_tensor_reduce (custom DVE ISA op) hangs this runtime -- all
row reductions use tensor_tensor + tensor_reduce or ACT accum_out instead.
"""

import numpy as np

B = 4096
TB = 2 * B      # 8192 rows of reps
D = 128
TAU = 0.5
N_CORES = 8
R = TB // N_CORES   # 1024 rows per core
MT = R // 128       # 8 row-tiles owned per core
NT = TB // 128      # 64 row-tiles total
G = 4               # column supergroups
GT = NT // G        # 16 row-tiles per supergroup
GC = TB // G        # 2048 columns per supergroup

MAGIC = 0x5F3759DF  # fast inverse-sqrt initial guess

_CACHE = {}


def _build():
    import concourse.tile as tile
    from concourse import bacc, mybir

    f32 = mybir.dt.float32
    f32r = mybir.dt.float32r
    i32 = mybir.dt.int32
    Exp = mybir.ActivationFunctionType.Exp
    OpAdd = mybir.AluOpType.add
    OpMult = mybir.AluOpType.mult
    OpShr = mybir.AluOpType.arith_shift_right
    OpXor = mybir.AluOpType.bitwise_xor
    AxisX = mybir.AxisListType.X

    nc = bacc.Bacc(
        "TRN2", target_bir_lowering=False, debug=False, num_devices=N_CORES
    )
    e_ap = nc.dram_tensor("e", [TB, D], f32, kind="ExternalInput").ap()
    ident_ap = nc.dram_tensor("ident", [128, 128], f32, kind="ExternalInput").ap()
    rs_ap = nc.dram_tensor("rs", [128, MT], f32, kind="ExternalOutput").ap()
    pos_ap = nc.dram_tensor("pos", [128, MT], f32, kind="ExternalOutput").ap()
    z2_ap = nc.dram_tensor("z2", [128, MT], f32, kind="ExternalOutput").ap()

    with tile.TileContext(nc) as tc:
        with (
            tc.tile_pool(name="xp", bufs=1) as xp,
            tc.tile_pool(name="ztp", bufs=1) as ztp,
            tc.tile_pool(name="small", bufs=1) as sp,
            tc.tile_pool(name="sq", bufs=2) as sqp,
            tc.tile_pool(name="ps", bufs=2, space="PSUM") as pp,
        ):
            ident = sp.tile([128, 128], f32, tag="ident")
            nc.scalar.dma_start(ident[:], ident_ap[:])
            # Dummy exp right after the ident load: pulls the one ACT
            # table load off the critical path (overlaps input DMA).
            dummy = sp.tile([128, 1], f32, tag="dummy")
            nc.scalar.activation(dummy[:], ident[:, 0:1], Exp)

            # Raw rows: one [128, 16*128] tile per supergroup, loaded with a
            # single strided DMA (HWDGE queues alternate between groups).
            # Scaled in place to Z after normalization.
            dma_engines = [nc.sync, nc.scalar]
            xgs = []
            for g in range(G):
                xg = xp.tile([128, GC], f32, tag=f"xg{g}", name=f"xg{g}")
                src = e_ap[g * GC : (g + 1) * GC, :].rearrange(
                    "(j p) d -> p j d", p=128
                )
                dst = xg[:].rearrange("p (j d) -> p j d", d=128)
                dma_engines[g % 2].dma_start(dst, src)
                xgs.append(xg)

            def xtile(t):
                g, j = divmod(t, GT)
                return xgs[g][:, j * 128 : (j + 1) * 128]

            s2 = sp.tile([128, NT], f32, tag="s2")
            inv = sp.tile([128, NT], f32, tag="inv")
            nrt = sp.tile([128, NT], f32, tag="nrt")
            parts = sp.tile([128, MT * G], f32, tag="parts")
            rs_t = sp.tile([128, MT], f32, tag="rs")
            pos_t = sp.tile([128, MT], f32, tag="pos")
            z2_t = sp.tile([128, MT], f32, tag="z2")
            inv2 = sp.tile([128, MT], f32, tag="inv2")

            # Transposed normalized rows, one [128(d), 2048(rows)] tile per
            # supergroup, rounded to float32r for the PE's single-pass fp32
            # matmul mode (the DVE copy out of PSUM performs the rounding).
            # ZT group 0 also holds this core's own 1024 rows.
            zts = [
                ztp.tile([128, GC], f32r, tag=f"zt{g}", name=f"zt{g}")
                for g in range(G)
            ]

            def rsqrt(cols):
                """inv[:, cols] = 1/sqrt(s2[:, cols]) via magic guess + two
                Newton steps, entirely on DVE (no ACT table switches).
                MAGIC - x == ~x + (MAGIC+1) avoids a reverse-subtract op."""
                s2i = s2[:, cols].bitcast(i32)
                invi = inv[:, cols].bitcast(i32)
                nc.vector.tensor_scalar(
                    out=invi, in0=s2i, scalar1=1, scalar2=-1,
                    op0=OpShr, op1=OpXor,
                )
                nc.vector.tensor_scalar(
                    out=invi, in0=invi, scalar1=MAGIC + 1, scalar2=None, op0=OpAdd
                )
                for _ in range(2):
                    nr = nrt[:, cols]
                    nc.vector.tensor_tensor(nr, inv[:, cols], inv[:, cols], OpMult)
                    nc.vector.tensor_tensor(nr, nr, s2[:, cols], OpMult)
                    nc.vector.tensor_scalar(
                        out=nr, in0=nr, scalar1=-0.5, scalar2=1.5,
                        op0=OpMult, op1=OpAdd,
                    )
                    nc.vector.tensor_tensor(inv[:, cols], inv[:, cols], nr, OpMult)

            def prep_group(g):
                gcols = slice(g * GT, (g + 1) * GT)
                # squares -> batched row-reduce -> s2 for the group's tiles.
                # Overlapped groups (g>0) square on the otherwise-idle GpSimd
                # to leave DVE slack inside the ACT-paced main-loop window;
                # group 0 stays on the faster DVE (it gates the pipeline).
                sq_engine = nc.vector if g == 0 else nc.gpsimd
                sq = sqp.tile([128, GC], f32, tag="sq", name=f"sq{g}")
                for j in range(GT):
                    t = g * GT + j
                    sq_engine.tensor_tensor(
                        sq[:, j * 128 : (j + 1) * 128], xtile(t), xtile(t), OpMult
                    )
                sq3 = sq[:].rearrange("p (j d) -> p j d", d=128)
                nc.vector.tensor_reduce(s2[:, gcols], sq3, axis=AxisX, op=OpAdd)
                rsqrt(gcols)
                for j in range(GT):
                    t = g * GT + j
                    nc.vector.tensor_scalar_mul(xtile(t), xtile(t), inv[:, t : t + 1])
                tp = pp.tile([128, GC], f32, tag="ps", name=f"tp{g}")
                for j in range(GT):
                    t = g * GT + j
                    nc.tensor.transpose(tp[:, j * 128 : (j + 1) * 128], xtile(t), ident[:])
                # Chunked copy-out (DVE; DMA cannot read PSUM) so the PSUM
                # slot frees progressively. Converts fp32 -> float32r.
                for j in range(4):
                    cols = slice(j * 512, (j + 1) * 512)
                    nc.vector.tensor_copy(zts[g][:, cols], tp[:, cols])

            def mm_group(g, m):
                mm = pp.tile([128, GC], f32, tag="ps", name=f"mm{g}_{m}")
                lhsT = zts[0][:, m * 128 : (m + 1) * 128]
                for j in range(4):
                    cols = slice(j * 512, (j + 1) * 512)
                    nc.tensor.matmul(mm[:, cols], lhsT, zts[g][:, cols])
                # exp in place in PSUM (discarded); accum_out is the row sum.
                idx = m * G + g
                nc.scalar.activation(
                    mm[:],
                    mm[:],
                    Exp,
                    scale=1.0 / TAU,
                    accum_out=parts[:, idx : idx + 1],
                )

            prep_group(0)
            for g in range(G):
                if g == 1:
                    # z2 (self-similarity of my rows) = s2 * inv^2, from
                    # group-0 values.
                    nc.vector.tensor_tensor(
                        inv2[:], inv[:, :MT], inv[:, :MT], OpMult
                    )
                    nc.vector.tensor_tensor(z2_t[:], inv2[:], s2[:, :MT], OpMult)
                if g == 3:
                    # Positive pairs: my local row l pairs with local row
                    # l + 4096 = tile m + 32 (holds for both halves under
                    # the rotation). Tiles 32..39 are scaled by prep_group(2).
                    psq = sqp.tile([128, MT * 128], f32, tag="sq", name="psq")
                    for m in range(MT):
                        nc.vector.tensor_tensor(
                            psq[:, m * 128 : (m + 1) * 128],
                            xtile(m),
                            xtile(m + NT // 2),
                            OpMult,
                        )
                    psq3 = psq[:].rearrange("p (m d) -> p m d", d=128)
                    nc.vector.tensor_reduce(pos_t[:], psq3, axis=AxisX, op=OpAdd)
                for m in range(MT):
                    if g < G - 1 and m == 3:
                        prep_group(g + 1)
                    mm_group(g, m)

            # rs = sum over the G partial sums per m-tile.
            parts3 = parts[:].rearrange("p (m g) -> p m g", g=G)
            nc.vector.tensor_reduce(rs_t[:], parts3, axis=AxisX, op=OpAdd)

            nc.sync.dma_start(rs_ap[:], rs_t[:])
            nc.sync.dma_start(pos_ap[:], pos_t[:])
            nc.sync.dma_start(z2_ap[:], z2_t[:])

    nc.compile()
    return nc


def _get_nc():
    if "nc" not in _CACHE:
        _CACHE["nc"] = _build()
    return _CACHE["nc"]


def kernel(e_i: np.ndarray, e_j: np.ndarray, _trace: bool = False):
    from concourse.bass_utils import run_bass_kernel_spmd

    nc = _get_nc()
    e = np.concatenate(
        [np.asarray(e_i, np.float32), np.asarray(e_j, np.float32)], axis=0
    )
    ident = np.eye(128, dtype=np.float32)
    in_maps = [
        {"e": np.ascontiguousarray(np.roll(e, -c * R, axis=0)), "ident": ident}
        for c in range(N_CORES)
    ]
    res = run_bass_kernel_spmd(nc, in_maps, list(range(N_CORES)), trace=_trace)
    _CACHE["last_exec_time_ns"] = res.exec_time_ns
    _CACHE["last_res"] = res

    rs = np.empty(TB, np.float64)
    z2 = np.empty(TB, np.float64)
    pos = np.empty(TB, np.float64)
    for c in range(N_CORES):
        o = res.results[c]
        rows = slice(c * R, (c + 1) * R)
        # out[p, m] is local row m*128+p -> transpose to row-major order.
        rs[rows] = o["rs"].astype(np.float64).T.reshape(-1)
        z2[rows] = o["z2"].astype(np.float64).T.reshape(-1)
        pos[rows] = o["pos"].astype(np.float64).T.reshape(-1)

    den = rs - np.exp(z2 / TAU)
    loss = np.mean(np.log(den) - pos / TAU)
    return np.float32(loss)


# revision 43
# speedup vs baseline: 1.1312x; 1.1312x over previous
"""NT-Xent loss Trainium2 kernel (8-core SPMD, Bass/Tile).

Math: loss = mean_a [ log(den_a) - pos_a/tau ],
  den_a = sum_{b != a} exp(sim_ab/tau),  sim = Z Z^T,  Z = row-normalized
  concat(e_i, e_j).

Sharding: row-parallel over the 8192 rows of the similarity matrix. Every
core receives the full embedding matrix rotated by -c*1024 rows so that its
1024 rows are always local rows 0..1023 (identical SPMD program on all
cores). Each core computes row sums of exp(sim/tau) for its rows against
all 8192 columns (fused exp+accumulate on the scalar engine), plus the
per-row self-similarity (z2) and positive-pair dot (pos). The host performs
the final gather: den = rowsum - exp(z2/tau), loss = mean(log den - pos/tau).

Engine budget per core: ACT does 8.4M exp (the bottleneck, ~66us); PE does
the 1024x8192x128 fp32r matmul plus 64 transposes; DVE does normalization
(squares, row reductions, a magic-constant+Newton rsqrt -- keeps ACT free
of Sqrt/Ln table loads), scaling, and the PSUM->SBUF float32r copies.

Note: tensor_tensor_reduce (custom DVE ISA op) hangs this runtime -- all
row reductions use tensor_tensor + tensor_reduce or ACT accum_out instead.
"""

import numpy as np

B = 4096
TB = 2 * B      # 8192 rows of reps
D = 128
TAU = 0.5
N_CORES = 8
R = TB // N_CORES   # 1024 rows per core
MT = R // 128       # 8 row-tiles owned per core
NT = TB // 128      # 64 row-tiles total
G = 4               # column supergroups
GT = NT // G        # 16 row-tiles per supergroup
GC = TB // G        # 2048 columns per supergroup

MAGIC = 0x5F3759DF  # fast inverse-sqrt initial guess

_CACHE = {}


def _build():
    import concourse.tile as tile
    from concourse import bacc, mybir

    f32 = mybir.dt.float32
    f32r = mybir.dt.float32r
    i32 = mybir.dt.int32
    Exp = mybir.ActivationFunctionType.Exp
    OpAdd = mybir.AluOpType.add
    OpMult = mybir.AluOpType.mult
    OpShr = mybir.AluOpType.arith_shift_right
    OpXor = mybir.AluOpType.bitwise_xor
    AxisX = mybir.AxisListType.X

    nc = bacc.Bacc(
        "TRN2", target_bir_lowering=False, debug=False, num_devices=N_CORES
    )
    e_ap = nc.dram_tensor("e", [TB, D], f32, kind="ExternalInput").ap()
    ident_ap = nc.dram_tensor("ident", [128, 128], f32, kind="ExternalInput").ap()
    rs_ap = nc.dram_tensor("rs", [128, MT], f32, kind="ExternalOutput").ap()
    pos_ap = nc.dram_tensor("pos", [128, MT], f32, kind="ExternalOutput").ap()
    z2_ap = nc.dram_tensor("z2", [128, MT], f32, kind="ExternalOutput").ap()

    with tile.TileContext(nc) as tc:
        with (
            tc.tile_pool(name="xp", bufs=1) as xp,
            tc.tile_pool(name="ztp", bufs=1) as ztp,
            tc.tile_pool(name="small", bufs=1) as sp,
            tc.tile_pool(name="sq", bufs=2) as sqp,
            tc.tile_pool(name="ps", bufs=2, space="PSUM") as pp,
        ):
            ident = sp.tile([128, 128], f32, tag="ident")
            nc.scalar.dma_start(ident[:], ident_ap[:])
            # Dummy exp right after the ident load: pulls the one ACT
            # table load off the critical path (overlaps input DMA).
            dummy = sp.tile([128, 1], f32, tag="dummy")
            nc.scalar.activation(dummy[:], ident[:, 0:1], Exp)

            # Raw rows: one [128, 16*128] tile per supergroup, loaded with a
            # single strided DMA (HWDGE queues alternate between groups).
            # Scaled in place to Z after normalization.
            dma_engines = [nc.sync, nc.scalar]
            xgs = []
            for g in range(G):
                xg = xp.tile([128, GC], f32, tag=f"xg{g}", name=f"xg{g}")
                src = e_ap[g * GC : (g + 1) * GC, :].rearrange(
                    "(j p) d -> p j d", p=128
                )
                dst = xg[:].rearrange("p (j d) -> p j d", d=128)
                dma_engines[g % 2].dma_start(dst, src)
                xgs.append(xg)

            def xtile(t):
                g, j = divmod(t, GT)
                return xgs[g][:, j * 128 : (j + 1) * 128]

            s2 = sp.tile([128, NT], f32, tag="s2")
            inv = sp.tile([128, NT], f32, tag="inv")
            nrt = sp.tile([128, NT], f32, tag="nrt")
            parts = sp.tile([128, MT * G], f32, tag="parts")
            rs_t = sp.tile([128, MT], f32, tag="rs")
            pos_t = sp.tile([128, MT], f32, tag="pos")
            z2_t = sp.tile([128, MT], f32, tag="z2")
            inv2 = sp.tile([128, MT], f32, tag="inv2")

            # Transposed normalized rows, one [128(d), 2048(rows)] tile per
            # supergroup, rounded to float32r for the PE's single-pass fp32
            # matmul mode (the DVE copy out of PSUM performs the rounding).
            # ZT group 0 also holds this core's own 1024 rows.
            zts = [
                ztp.tile([128, GC], f32r, tag=f"zt{g}", name=f"zt{g}")
                for g in range(G)
            ]

            def rsqrt(cols):
                """inv[:, cols] = 1/sqrt(s2[:, cols]) via magic guess + two
                Newton steps, entirely on DVE (no ACT table switches).
                MAGIC - x == ~x + (MAGIC+1) avoids a reverse-subtract op."""
                s2i = s2[:, cols].bitcast(i32)
                invi = inv[:, cols].bitcast(i32)
                nc.vector.tensor_scalar(
                    out=invi, in0=s2i, scalar1=1, scalar2=-1,
                    op0=OpShr, op1=OpXor,
                )
                nc.vector.tensor_scalar(
                    out=invi, in0=invi, scalar1=MAGIC + 1, scalar2=None, op0=OpAdd
                )
                for _ in range(2):
                    nr = nrt[:, cols]
                    nc.vector.tensor_tensor(nr, inv[:, cols], inv[:, cols], OpMult)
                    nc.vector.tensor_tensor(nr, nr, s2[:, cols], OpMult)
                    nc.vector.tensor_scalar(
                        out=nr, in0=nr, scalar1=-0.5, scalar2=1.5,
                        op0=OpMult, op1=OpAdd,
                    )
                    nc.vector.tensor_tensor(inv[:, cols], inv[:, cols], nr, OpMult)

            def prep_group(g):
                gcols = slice(g * GT, (g + 1) * GT)
                # squares -> batched row-reduce -> s2 for the group's tiles
                sq = sqp.tile([128, GC], f32, tag="sq", name=f"sq{g}")
                for j in range(GT):
                    t = g * GT + j
                    nc.vector.tensor_tensor(
                        sq[:, j * 128 : (j + 1) * 128], xtile(t), xtile(t), OpMult
                    )
                sq3 = sq[:].rearrange("p (j d) -> p j d", d=128)
                nc.vector.tensor_reduce(s2[:, gcols], sq3, axis=AxisX, op=OpAdd)
                rsqrt(gcols)
                for j in range(GT):
                    t = g * GT + j
                    nc.vector.tensor_scalar_mul(xtile(t), xtile(t), inv[:, t : t + 1])
                tp = pp.tile([128, GC], f32, tag="ps", name=f"tp{g}")
                for j in range(GT):
                    t = g * GT + j
                    nc.tensor.transpose(tp[:, j * 128 : (j + 1) * 128], xtile(t), ident[:])
                # Chunked copy-out (DVE; DMA cannot read PSUM) so the PSUM
                # slot frees progressively. Converts fp32 -> float32r.
                for j in range(4):
                    cols = slice(j * 512, (j + 1) * 512)
                    nc.vector.tensor_copy(zts[g][:, cols], tp[:, cols])

            def mm_group(g, m):
                mm = pp.tile([128, GC], f32, tag="ps", name=f"mm{g}_{m}")
                lhsT = zts[0][:, m * 128 : (m + 1) * 128]
                for j in range(4):
                    cols = slice(j * 512, (j + 1) * 512)
                    nc.tensor.matmul(mm[:, cols], lhsT, zts[g][:, cols])
                # exp in place in PSUM (discarded); accum_out is the row sum.
                idx = m * G + g
                nc.scalar.activation(
                    mm[:],
                    mm[:],
                    Exp,
                    scale=1.0 / TAU,
                    accum_out=parts[:, idx : idx + 1],
                )

            prep_group(0)
            for g in range(G):
                if g == 1:
                    # z2 (self-similarity of my rows) = s2 * inv^2, from
                    # group-0 values.
                    nc.vector.tensor_tensor(
                        inv2[:], inv[:, :MT], inv[:, :MT], OpMult
                    )
                    nc.vector.tensor_tensor(z2_t[:], inv2[:], s2[:, :MT], OpMult)
                if g == 3:
                    # Positive pairs: my local row l pairs with local row
                    # l + 4096 = tile m + 32 (holds for both halves under
                    # the rotation). Tiles 32..39 are scaled by prep_group(2).
                    psq = sqp.tile([128, MT * 128], f32, tag="sq", name="psq")
                    for m in range(MT):
                        nc.vector.tensor_tensor(
                            psq[:, m * 128 : (m + 1) * 128],
                            xtile(m),
                            xtile(m + NT // 2),
                            OpMult,
                        )
                    psq3 = psq[:].rearrange("p (m d) -> p m d", d=128)
                    nc.vector.tensor_reduce(pos_t[:], psq3, axis=AxisX, op=OpAdd)
                for m in range(MT):
                    if g < G - 1 and m == 3:
                        prep_group(g + 1)
                    mm_group(g, m)

            # rs = sum over the G partial sums per m-tile.
            parts3 = parts[:].rearrange("p (m g) -> p m g", g=G)
            nc.vector.tensor_reduce(rs_t[:], parts3, axis=AxisX, op=OpAdd)

            nc.sync.dma_start(rs_ap[:], rs_t[:])
            nc.sync.dma_start(pos_ap[:], pos_t[:])
            nc.sync.dma_start(z2_ap[:], z2_t[:])

    nc.compile()
    return nc


def _get_nc():
    if "nc" not in _CACHE:
        _CACHE["nc"] = _build()
    return _CACHE["nc"]


def kernel(e_i: np.ndarray, e_j: np.ndarray, _trace: bool = False):
    from concourse.bass_utils import run_bass_kernel_spmd

    nc = _get_nc()
    e = np.concatenate(
        [np.asarray(e_i, np.float32), np.asarray(e_j, np.float32)], axis=0
    )
    ident = np.eye(128, dtype=np.float32)
    in_maps = [
        {"e": np.ascontiguousarray(np.roll(e, -c * R, axis=0)), "ident": ident}
        for c in range(N_CORES)
    ]
    res = run_bass_kernel_spmd(nc, in_maps, list(range(N_CORES)), trace=_trace)
    _CACHE["last_exec_time_ns"] = res.exec_time_ns
    _CACHE["last_res"] = res

    rs = np.empty(TB, np.float64)
    z2 = np.empty(TB, np.float64)
    pos = np.empty(TB, np.float64)
    for c in range(N_CORES):
        o = res.results[c]
        rows = slice(c * R, (c + 1) * R)
        # out[p, m] is local row m*128+p -> transpose to row-major order.
        rs[rows] = o["rs"].astype(np.float64).T.reshape(-1)
        z2[rows] = o["z2"].astype(np.float64).T.reshape(-1)
        pos[rows] = o["pos"].astype(np.float64).T.reshape(-1)

    den = rs - np.exp(z2 / TAU)
    loss = np.mean(np.log(den) - pos / TAU)
    return np.float32(loss)
